# revision 22
# baseline (speedup 1.0000x reference)
# RWKV token-shift + LoRA mixing block for Trainium2, 8-core SPMD.
#
# Reference computation (per batch b, token t):
#   sx[t]     = x[t-1] - x[t]            (x[-1] comes from state row i1)
#   xk        = x + sx * time_maa_x
#   h         = tanh(xk @ w1)            # [T, 160]
#   xxx[f]    = h[:, 32f:32f+32] @ w2[f] # [T, D] for f in 0..4
#   out[t,f]  = x[t] + sx[t] * (maa_f + xxx[t,f])
#   new_state = state with row i1 := x[:, -1]
#
# Sharding: 8 cores = (batch b = c//2) x (sequence half = c%2), 1024 tokens
# per core.  The halo token (t0-1) for every 128-token tile is passed in as a
# host-prepared "halos" row so no cross-tile dependencies exist on device.
#
# Device strategy per core (T=1024, D=2048):
#  - sx via PE:  psum = (S - I) @ x_tile  (+ rank-1 e0 (x) halo row), ACT copy
#    to SBUF.  Avoids any cross-partition shifts on vector engines.
#  - x tiles are PE-transposed (128x128 blocks) into a resident xT [128,1+T]
#    bf16 per d-chunk; the "+1" halo column makes x[t-1] a free-dim offset.
#  - stage 1 (hT = tanh(xk @ w1)) uses the identity
#        xk @ w1 = x @ (w1 - tmx*w1) + x_prev @ (tmx*w1)
#    with both weight halves host-packed (bf16), so no elementwise prep at
#    all:  hT[j, t] accumulates 32 matmuls over d-chunks into PSUM, tanh on
#    ACT into SBUF (partition-aligned for stage 2 row groups).
#  - stage 2: per (tile, f): psum[tok, d] = ones (x) maa_f  +  hT_f^T @ w2_f,
#    row-group packed (f=0..3 at array rows 32f) so pairs run concurrently.
#  - final: DVE  out = psum * sx   (only DVE can TT with PSUM), then the
#    "+ x" add is split DVE/GPSIMD (GPSIMD cannot touch PSUM but adds SBUF
#    tiles fine), then HWDGE store straight into out[t, f, :].
import sys

if "/opt/trn_rl_repo" not in sys.path:
    sys.path.insert(0, "/opt/trn_rl_repo")

import numpy as np
import ml_dtypes

import concourse.bass as bass
import concourse.mybir as mybir
from concourse import bacc, tile
from concourse.bass_utils import run_bass_kernel_spmd

F32 = mybir.dt.float32
BF16 = mybir.dt.bfloat16
TANH = mybir.ActivationFunctionType.Tanh

D = 2048
J = 160
NK = D // 128  # 16 d-chunks
HEAD = 64
N_CORES = 8

LAST_RESULT = None  # BassKernelResults of the most recent run (for profiling)


def build_tile_program(tc, io, T, tps=2):
    """Emit the per-core tile program.

    io: dict name -> bass.AP for dram tensors.
    T: tokens handled by this core.  tps: 128-token tiles per super-tile.
    """
    from contextlib import ExitStack
    ctx = ExitStack()
    nc = tc.nc
    nt = T // 128
    assert nt % tps == 0
    N = tps * 128  # stage-1/2 token block width

    x_d, out_d = io["x"], io["out"]

    wts = ctx.enter_context(tc.tile_pool(name="wts", bufs=1))
    xp = ctx.enter_context(tc.tile_pool(name="xp", bufs=min(nt, 5)))
    sxp = ctx.enter_context(tc.tile_pool(name="sxp", bufs=min(nt, tps + 2)))
    xtp = ctx.enter_context(tc.tile_pool(name="xtp", bufs=NK))
    htp = ctx.enter_context(tc.tile_pool(name="htp", bufs=2))
    outp = ctx.enter_context(tc.tile_pool(name="outp", bufs=5))
    pss = ctx.enter_context(
        tc.tile_pool(name="pss", bufs=4, space=bass.MemorySpace.PSUM))
    pso = ctx.enter_context(
        tc.tile_pool(name="pso", bufs=2, space=bass.MemorySpace.PSUM))

    def load_const(name, shape, dtype):
        t = wts.tile(shape, dtype, tag=name, name=name + "_sb")
        nc.sync.dma_start(t[:], io[name][:])
        return t

    wpack = load_const("wpack", [128, 2 * NK * J], BF16)
    w2p = load_const("w2pack", [128, D], F32)
    w24 = load_const("w24", [32, D], F32)
    maap = load_const("maapack", [128, D], BF16)
    maag = load_const("maag", [1, D], BF16)
    shiftm = load_const("shiftm", [128, 128], F32)
    e0 = load_const("e0", [1, 128], F32)
    e127 = load_const("e127", [128, 128], F32)
    ones = load_const("ones", [128, 128], BF16)
    halo0 = load_const("halo0", [1, D], F32)
    haloT = load_const("haloT", [128, NK], BF16)

    # Resident transposed x, one [128, 16+T] bf16 tile per 128-wide d-chunk.
    # Column 16+t is token t; column 15 is the halo token (t = -1); cols
    # 0-14 pad so token columns stay 32-byte aligned for the xbar DMA.
    # Each tile is filled by ONE DMA-transpose from the host-cast bf16 x.
    xT = [xtp.tile([128, 16 + T], BF16, tag="xT", name=f"xT{k}")
          for k in range(NK)]
    xbf_d = io["xbf"]
    for k in range(NK):
        nc.sync.dma_start(xT[k][:, 16:16 + T],
                          xbf_d[:, k * 128:(k + 1) * 128], transpose=True)
        nc.scalar.copy(xT[k][:, 15:16], haloT[:, k:k + 1])

    x_tiles = [None] * nt
    sx_tiles = [None] * nt

    for i in range(nt):
        xi = xp.tile([128, D], F32, tag="x")
        nc.sync.dma_start(xi[:], x_d[i * 128:(i + 1) * 128, :])
        x_tiles[i] = xi

        # sx = (S - I) @ x + halo into row 0   (PE), then ACT copy to SBUF.
        # Tile 0's halo is the DRAM halo row (rank-1 via e0); later tiles
        # take row 127 of the previous x tile (one-hot e127 matmul).
        sxi = sxp.tile([128, D], F32, tag="sx")
        for c in range(4):
            ps = pss.tile([128, 512], F32, tag="ps")
            cs = slice(c * 512, (c + 1) * 512)
            nc.tensor.matmul(ps[:], shiftm[:], xi[:, cs], start=True, stop=False)
            if i == 0:
                nc.tensor.matmul(ps[:], e0[:], halo0[0:1, cs],
                                 start=False, stop=True)
            else:
                nc.tensor.matmul(ps[:], e127[:], x_tiles[i - 1][:, cs],
                                 start=False, stop=True)
            nc.scalar.copy(sxi[:, cs], ps[:])
        sx_tiles[i] = sxi

        if i % tps != tps - 1:
            continue

        # ---- super-tile s complete: stage 1 then stage 2 ----
        s = i // tps

        ph = pss.tile([128, N], F32, tag="ps")
        ph2 = pss.tile([32, N], F32, tag="ps")
        for (mo, msz, pt_) in ((0, 128, ph), (128, 32, ph2)):
            nmm = 0
            for pass_ in range(2):      # 0: w1b against x,  1: w1p against x_prev
                off = 16 - pass_        # halo-padded column offset
                for k in range(NK):
                    c0 = (pass_ * NK + k) * J + mo
                    nc.tensor.matmul(
                        pt_[:],
                        wpack[:, c0:c0 + msz],
                        xT[k][:, s * N + off: s * N + off + N],
                        start=(nmm == 0), stop=(nmm == 2 * NK - 1))
                    nmm += 1
        hts = htp.tile([128, N], F32, tag="ht")
        ht2 = htp.tile([32, N], F32, tag="ht2")
        nc.scalar.activation(hts[:], ph[:], TANH)
        nc.scalar.activation(ht2[:], ph2[:], TANH)

        for ii in range(s * tps, (s + 1) * tps):
            tloc = (ii % tps) * 128
            outs = [outp.tile([128, D], F32, tag="out", name=f"out{ii}_{f}")
                    for f in range(5)]
            for h in range(2):
                hs = slice(h * 1024, (h + 1) * 1024)
                for pair in ((0, 1), (2, 3), (4,)):
                    pos = {f: pso.tile([128, 1024], F32, tag="po", name=f"po{f}")
                           for f in pair}
                    for f in pair:
                        if f < 4:
                            l_r1 = ones[32 * f:32 * f + 1, 0:128]
                            l_mm = hts[32 * f:32 * f + 32, tloc:tloc + 128]
                            tp = (96, 0) if f == 3 else None
                        else:
                            l_r1 = ones[0:1, 0:128]
                            l_mm = ht2[:, tloc:tloc + 128]
                            tp = None
                        for c in (2 * h, 2 * h + 1):
                            sl = slice((c % 2) * 512, (c % 2) * 512 + 512)
                            cs = slice(c * 512, (c + 1) * 512)
                            r_r1 = maap[32 * f:32 * f + 1, cs] if f < 4 \
                                else maag[0:1, cs]
                            r_mm = w2p[32 * f:32 * f + 32, cs] if f < 4 \
                                else w24[:, cs]
                            nc.tensor.matmul(pos[f][:, sl], l_r1, r_r1,
                                             start=True, stop=False,
                                             tile_position=tp)
                            nc.tensor.matmul(pos[f][:, sl], l_mm, r_mm,
                                             start=False, stop=True,
                                             tile_position=tp)
                    for f in pair:
                        nc.vector.tensor_mul(outs[f][:, hs], pos[f][:],
                                             sx_tiles[ii][:, hs])
            for f in range(5):
                # ~1/3 of the adds stay on DVE, the rest go to GPSIMD so the
                # PSUM-bound muls (DVE-only) and adds overlap.
                eng = nc.vector if (ii * 5 + f) % 3 == 0 else nc.gpsimd
                eng.tensor_add(outs[f][:], outs[f][:], x_tiles[ii][:])
                # Stores ride the ACT HWDGE ring; loads/transposes use the
                # sync ring — two rings double the DMA issue bandwidth.
                nc.scalar.dma_start(out_d[ii * 128:(ii + 1) * 128, f, :],
                                    outs[f][:])
    ctx.close()


def host_pack_weights(time_maa_x, time_maa_w1, time_maa_w2,
                      maa_k, maa_w, maa_v, maa_r, maa_g):
    tmx = np.asarray(time_maa_x, np.float32)
    w1 = np.asarray(time_maa_w1, np.float32)
    w2 = np.asarray(time_maa_w2, np.float32)
    w1p = w1 * tmx[:, None]
    w1b = w1 - w1p
    # wpack[p, pass, k, j] = w1x[pass][k*128+p, j]
    wpack = (np.stack([w1b, w1p], 0)
             .reshape(2, NK, 128, J)
             .transpose(2, 0, 1, 3)
             .reshape(128, 2 * NK * J)
             .astype(ml_dtypes.bfloat16))
    maas = np.stack([maa_k, maa_w, maa_v, maa_r, maa_g]).astype(np.float32)
    maapack = np.zeros((128, D), np.float32)
    maapack[[0, 32, 64, 96]] = maas[:4]
    e127 = np.zeros((128, 128), np.float32)
    e127[127, 0] = 1.0
    consts = {
        "wpack": wpack,
        "w2pack": np.ascontiguousarray(w2[:4].reshape(128, D), dtype=np.float32),
        "w24": np.ascontiguousarray(w2[4], dtype=np.float32),
        "maapack": maapack.astype(ml_dtypes.bfloat16),
        "maag": np.ascontiguousarray(maas[4:5]).astype(ml_dtypes.bfloat16),
        "shiftm": (np.eye(128, 128, 1) - np.eye(128)).astype(np.float32),
        "e0": np.eye(1, 128, dtype=np.float32),
        "e127": e127,
        "ones": np.ones((128, 128), ml_dtypes.bfloat16),
    }
    return consts


def build_nc(T, tps=2):
    # Bacc (not bare Bass): its compile() runs move_matmul_waits_to_ldweights
    # and generate_event_semaphores, which split multi-semaphore waits to
    # satisfy the 1-wait-per-instruction TRN2 constraint walrus enforces.
    nc = bacc.Bacc("TRN2", target_bir_lowering=False, debug=False)
    nt = T // 128
    io = {
        "x": nc.dram_tensor("x", [T, D], F32, kind="ExternalInput").ap(),
        "xbf": nc.dram_tensor("xbf", [T, D], BF16, kind="ExternalInput").ap(),
        "halo0": nc.dram_tensor("halo0", [1, D], F32,
                                kind="ExternalInput").ap(),
        "haloT": nc.dram_tensor("haloT", [128, NK], BF16,
                                kind="ExternalInput").ap(),
        "wpack": nc.dram_tensor("wpack", [128, 2 * NK * J], BF16,
                                kind="ExternalInput").ap(),
        "w2pack": nc.dram_tensor("w2pack", [128, D], F32,
                                 kind="ExternalInput").ap(),
        "w24": nc.dram_tensor("w24", [32, D], F32, kind="ExternalInput").ap(),
        "maapack": nc.dram_tensor("maapack", [128, D], BF16,
                                  kind="ExternalInput").ap(),
        "maag": nc.dram_tensor("maag", [1, D], BF16,
                               kind="ExternalInput").ap(),
        "shiftm": nc.dram_tensor("shiftm", [128, 128], F32,
                                 kind="ExternalInput").ap(),
        "e0": nc.dram_tensor("e0", [1, 128], F32, kind="ExternalInput").ap(),
        "e127": nc.dram_tensor("e127", [128, 128], F32,
                               kind="ExternalInput").ap(),
        "ones": nc.dram_tensor("ones", [128, 128], BF16,
                               kind="ExternalInput").ap(),
        "out": nc.dram_tensor("out", [T, 5, D], F32,
                              kind="ExternalOutput").ap(),
    }
    with tile.TileContext(nc) as tc:
        build_tile_program(tc, io, T, tps)
    nc.compile()
    return nc


_NC_CACHE = {}


def kernel(x, state, time_maa_x, time_maa_w1, time_maa_w2,
           maa_k, maa_w, maa_v, maa_r, maa_g, i):
    global LAST_RESULT
    x = np.asarray(x, np.float32)
    state = np.asarray(state, np.float32)
    B, S, D_ = x.shape
    assert (B, S, D_) == (4, 2048, D)
    T = S * B // N_CORES  # 1024 tokens per core
    i1 = (2 + HEAD) * int(i) + 1

    consts = host_pack_weights(time_maa_x, time_maa_w1, time_maa_w2,
                               maa_k, maa_w, maa_v, maa_r, maa_g)
    in_maps = []
    for c in range(N_CORES):
        b, half = c // 2, c % 2
        xs = np.ascontiguousarray(x[b, half * T:(half + 1) * T])
        prev = state[b, i1] if half == 0 else x[b, T - 1]
        in_maps.append({
            "x": xs,
            "xbf": xs.astype(ml_dtypes.bfloat16),
            "halo0": np.ascontiguousarray(prev[None], np.float32),
            "haloT": np.ascontiguousarray(
                prev.reshape(NK, 128).T).astype(ml_dtypes.bfloat16),
            **consts})

    key = T
    if key not in _NC_CACHE:
        _NC_CACHE[key] = build_nc(T)
    nc = _NC_CACHE[key]

    global _LAST_IN_MAPS
    _LAST_IN_MAPS = in_maps
    LAST_RESULT = run_bass_kernel_spmd(nc, in_maps, list(range(N_CORES)))
    res = LAST_RESULT.results

    out = np.empty((B, S, 5, D), np.float32)
    for c in range(N_CORES):
        b, half = c // 2, c % 2
        out[b, half * T:(half + 1) * T] = res[c]["out"].reshape(T, 5, D)

    new_state = state.copy()
    new_state[:, i1] = x[:, -1]
    return out, new_state


# revision 25
# speedup vs baseline: 1.4495x; 1.4495x over previous
# RWKV token-shift + LoRA mixing block for Trainium2, 8-core SPMD.
#
# Reference computation (per batch b, token t):
#   sx[t]     = x[t-1] - x[t]            (x[-1] comes from state row i1)
#   xk        = x + sx * time_maa_x
#   h         = tanh(xk @ w1)            # [T, 160]
#   xxx[f]    = h[:, 32f:32f+32] @ w2[f] # [T, D] for f in 0..4
#   out[t,f]  = x[t] + sx[t] * (maa_f + xxx[t,f])
#   new_state = state with row i1 := x[:, -1]
#
# Sharding: 8 cores = (batch b = c//2) x (sequence half = c%2), 1024 tokens
# per core.  The halo token (t0-1) for every 128-token tile is passed in as a
# host-prepared "halos" row so no cross-tile dependencies exist on device.
#
# Device strategy per core (T=1024, D=2048):
#  - sx via PE:  psum = (S - I) @ x_tile  (+ rank-1 e0 (x) halo row), ACT copy
#    to SBUF.  Avoids any cross-partition shifts on vector engines.
#  - x tiles are PE-transposed (128x128 blocks) into a resident xT [128,1+T]
#    bf16 per d-chunk; the "+1" halo column makes x[t-1] a free-dim offset.
#  - stage 1 (hT = tanh(xk @ w1)) uses the identity
#        xk @ w1 = x @ (w1 - tmx*w1) + x_prev @ (tmx*w1)
#    with both weight halves host-packed (bf16), so no elementwise prep at
#    all:  hT[j, t] accumulates 32 matmuls over d-chunks into PSUM, tanh on
#    ACT into SBUF (partition-aligned for stage 2 row groups).
#  - stage 2: per (tile, f): psum[tok, d] = ones (x) maa_f  +  hT_f^T @ w2_f,
#    row-group packed (f=0..3 at array rows 32f) so pairs run concurrently.
#  - final: DVE  out = psum * sx   (only DVE can TT with PSUM), then the
#    "+ x" add is split DVE/GPSIMD (GPSIMD cannot touch PSUM but adds SBUF
#    tiles fine), then HWDGE store straight into out[t, f, :].
import sys

if "/opt/trn_rl_repo" not in sys.path:
    sys.path.insert(0, "/opt/trn_rl_repo")

import numpy as np
import ml_dtypes

import concourse.bass as bass
import concourse.mybir as mybir
from concourse import bacc, tile
from concourse.bass_utils import run_bass_kernel_spmd

F32 = mybir.dt.float32
F32R = mybir.dt.float32r  # fp32 bits, reduced-precision multiply, 4x faster PE
BF16 = mybir.dt.bfloat16
TANH = mybir.ActivationFunctionType.Tanh

D = 2048
J = 160
NK = D // 128  # 16 d-chunks
HEAD = 64
N_CORES = 8

LAST_RESULT = None  # BassKernelResults of the most recent run (for profiling)


def build_tile_program(tc, io, T, tps=2):
    """Emit the per-core tile program.

    io: dict name -> bass.AP for dram tensors.
    T: tokens handled by this core.  tps: 128-token tiles per super-tile.
    """
    from contextlib import ExitStack
    ctx = ExitStack()
    nc = tc.nc
    nt = T // 128
    assert nt % tps == 0
    N = tps * 128  # stage-1/2 token block width

    x_d, out_d = io["x"], io["out"]

    wts = ctx.enter_context(tc.tile_pool(name="wts", bufs=1))
    xp = ctx.enter_context(tc.tile_pool(name="xp", bufs=min(nt, 5)))
    sxp = ctx.enter_context(tc.tile_pool(name="sxp", bufs=min(nt, tps + 2)))
    xtp = ctx.enter_context(tc.tile_pool(name="xtp", bufs=NK))
    htp = ctx.enter_context(tc.tile_pool(name="htp", bufs=2))
    outp = ctx.enter_context(tc.tile_pool(name="outp", bufs=5))
    pss = ctx.enter_context(
        tc.tile_pool(name="pss", bufs=4, space=bass.MemorySpace.PSUM))
    pso = ctx.enter_context(
        tc.tile_pool(name="pso", bufs=2, space=bass.MemorySpace.PSUM))

    def load_const(name, shape, dtype):
        t = wts.tile(shape, dtype, tag=name, name=name + "_sb")
        nc.sync.dma_start(t[:], io[name][:].bitcast(dtype))
        return t

    wpack = load_const("wpack", [128, 2 * NK * J], BF16)
    w2p = load_const("w2pack", [128, D], F32R)
    w24 = load_const("w24", [32, D], F32R)
    maap = load_const("maapack", [128, D], BF16)
    maag = load_const("maag", [1, D], BF16)
    shiftm = load_const("shiftm", [128, 128], F32R)
    e0 = load_const("e0", [1, 128], F32R)
    e127 = load_const("e127", [128, 128], F32R)
    ones = load_const("ones", [128, 128], BF16)
    halo0 = load_const("halo0", [1, D], F32R)
    haloT = load_const("haloT", [128, NK], BF16)

    # Resident transposed x, one [128, 16+T] bf16 tile per 128-wide d-chunk.
    # Column 16+t is token t; column 15 is the halo token (t = -1); cols
    # 0-14 pad so token columns stay 32-byte aligned for the xbar DMA.
    # Each tile is filled by ONE DMA-transpose from the host-cast bf16 x.
    xT = [xtp.tile([128, 16 + T], BF16, tag="xT", name=f"xT{k}")
          for k in range(NK)]
    xbf_d = io["xbf"]
    for k in range(NK):
        nc.sync.dma_start(xT[k][:, 16:16 + T],
                          xbf_d[:, k * 128:(k + 1) * 128], transpose=True)
        nc.scalar.copy(xT[k][:, 15:16], haloT[:, k:k + 1])

    x_tiles = [None] * nt
    sx_tiles = [None] * nt

    for i in range(nt):
        xi = xp.tile([128, D], F32R, tag="x")
        nc.sync.dma_start(xi[:], x_d[i * 128:(i + 1) * 128, :].bitcast(F32R))
        x_tiles[i] = xi

        # sx = (S - I) @ x + halo into row 0   (PE), then ACT copy to SBUF.
        # Tile 0's halo is the DRAM halo row (rank-1 via e0); later tiles
        # take row 127 of the previous x tile (one-hot e127 matmul).
        sxi = sxp.tile([128, D], F32, tag="sx")
        for c in range(4):
            ps = pss.tile([128, 512], F32, tag="ps")
            cs = slice(c * 512, (c + 1) * 512)
            nc.tensor.matmul(ps[:], shiftm[:], xi[:, cs],
                             start=True, stop=False)
            if i == 0:
                nc.tensor.matmul(ps[:], e0[:], halo0[0:1, cs],
                                 start=False, stop=True)
            else:
                nc.tensor.matmul(ps[:], e127[:], x_tiles[i - 1][:, cs],
                                 start=False, stop=True)
            nc.scalar.copy(sxi[:, cs], ps[:])
        sx_tiles[i] = sxi

        if i % tps != tps - 1:
            continue

        # ---- super-tile s complete: stage 1 then stage 2 ----
        s = i // tps

        ph = pss.tile([128, N], F32, tag="ps")
        ph2 = pss.tile([32, N], F32, tag="ps")
        for (mo, msz, pt_) in ((0, 128, ph), (128, 32, ph2)):
            nmm = 0
            for pass_ in range(2):      # 0: w1b against x,  1: w1p against x_prev
                off = 16 - pass_        # halo-padded column offset
                for k in range(NK):
                    c0 = (pass_ * NK + k) * J + mo
                    nc.tensor.matmul(
                        pt_[:],
                        wpack[:, c0:c0 + msz],
                        xT[k][:, s * N + off: s * N + off + N],
                        start=(nmm == 0), stop=(nmm == 2 * NK - 1))
                    nmm += 1
        hts = htp.tile([128, N], F32R, tag="ht")
        ht2 = htp.tile([32, N], F32R, tag="ht2")
        nc.scalar.activation(hts[:], ph[:], TANH)
        nc.scalar.activation(ht2[:], ph2[:], TANH)

        for ii in range(s * tps, (s + 1) * tps):
            tloc = (ii % tps) * 128
            outs = [outp.tile([128, D], F32, tag="out", name=f"out{ii}_{f}")
                    for f in range(5)]
            for h in range(2):
                hs = slice(h * 1024, (h + 1) * 1024)
                for pair in ((0, 1), (2, 3), (4,)):
                    pos = {f: pso.tile([128, 1024], F32, tag="po", name=f"po{f}")
                           for f in pair}
                    for f in pair:
                        if f < 4:
                            l_r1 = ones[32 * f:32 * f + 1, 0:128]
                            l_mm = hts[32 * f:32 * f + 32, tloc:tloc + 128]
                            tp = (96, 0) if f == 3 else None
                        else:
                            l_r1 = ones[0:1, 0:128]
                            l_mm = ht2[:, tloc:tloc + 128]
                            tp = None
                        for c in (2 * h, 2 * h + 1):
                            sl = slice((c % 2) * 512, (c % 2) * 512 + 512)
                            cs = slice(c * 512, (c + 1) * 512)
                            r_r1 = maap[32 * f:32 * f + 1, cs] if f < 4 \
                                else maag[0:1, cs]
                            r_mm = w2p[32 * f:32 * f + 32, cs] if f < 4 \
                                else w24[:, cs]
                            nc.tensor.matmul(pos[f][:, sl], l_r1, r_r1,
                                             start=True, stop=False,
                                             tile_position=tp)
                            nc.tensor.matmul(pos[f][:, sl], l_mm, r_mm,
                                             start=False, stop=True,
                                             tile_position=tp)
                    for f in pair:
                        nc.vector.tensor_mul(outs[f][:, hs], pos[f][:],
                                             sx_tiles[ii][:, hs])
            for f in range(5):
                # ~1/3 of the adds stay on DVE, the rest go to GPSIMD so the
                # PSUM-bound muls (DVE-only) and adds overlap.
                eng = nc.vector if (ii * 5 + f) % 3 == 0 else nc.gpsimd
                eng.tensor_add(outs[f][:], outs[f][:],
                               x_tiles[ii][:].bitcast(F32))
                # Stores ride the ACT HWDGE ring; loads/transposes use the
                # sync ring — two rings double the DMA issue bandwidth.
                nc.scalar.dma_start(out_d[ii * 128:(ii + 1) * 128, f, :],
                                    outs[f][:])
    ctx.close()


def host_pack_weights(time_maa_x, time_maa_w1, time_maa_w2,
                      maa_k, maa_w, maa_v, maa_r, maa_g):
    tmx = np.asarray(time_maa_x, np.float32)
    w1 = np.asarray(time_maa_w1, np.float32)
    w2 = np.asarray(time_maa_w2, np.float32)
    w1p = w1 * tmx[:, None]
    w1b = w1 - w1p
    # wpack[p, pass, k, j] = w1x[pass][k*128+p, j]
    wpack = (np.stack([w1b, w1p], 0)
             .reshape(2, NK, 128, J)
             .transpose(2, 0, 1, 3)
             .reshape(128, 2 * NK * J)
             .astype(ml_dtypes.bfloat16))
    maas = np.stack([maa_k, maa_w, maa_v, maa_r, maa_g]).astype(np.float32)
    maapack = np.zeros((128, D), np.float32)
    maapack[[0, 32, 64, 96]] = maas[:4]
    e127 = np.zeros((128, 128), np.float32)
    e127[127, 0] = 1.0
    consts = {
        "wpack": wpack,
        "w2pack": np.ascontiguousarray(w2[:4].reshape(128, D), dtype=np.float32),
        "w24": np.ascontiguousarray(w2[4], dtype=np.float32),
        "maapack": maapack.astype(ml_dtypes.bfloat16),
        "maag": np.ascontiguousarray(maas[4:5]).astype(ml_dtypes.bfloat16),
        "shiftm": (np.eye(128, 128, 1) - np.eye(128)).astype(np.float32),
        "e0": np.eye(1, 128, dtype=np.float32),
        "e127": e127,
        "ones": np.ones((128, 128), ml_dtypes.bfloat16),
    }
    return consts


def build_nc(T, tps=2):
    # Bacc (not bare Bass): its compile() runs move_matmul_waits_to_ldweights
    # and generate_event_semaphores, which split multi-semaphore waits to
    # satisfy the 1-wait-per-instruction TRN2 constraint walrus enforces.
    nc = bacc.Bacc("TRN2", target_bir_lowering=False, debug=False)
    nt = T // 128
    io = {
        "x": nc.dram_tensor("x", [T, D], F32, kind="ExternalInput").ap(),
        "xbf": nc.dram_tensor("xbf", [T, D], BF16, kind="ExternalInput").ap(),
        "halo0": nc.dram_tensor("halo0", [1, D], F32,
                                kind="ExternalInput").ap(),
        "haloT": nc.dram_tensor("haloT", [128, NK], BF16,
                                kind="ExternalInput").ap(),
        "wpack": nc.dram_tensor("wpack", [128, 2 * NK * J], BF16,
                                kind="ExternalInput").ap(),
        "w2pack": nc.dram_tensor("w2pack", [128, D], F32,
                                 kind="ExternalInput").ap(),
        "w24": nc.dram_tensor("w24", [32, D], F32, kind="ExternalInput").ap(),
        "maapack": nc.dram_tensor("maapack", [128, D], BF16,
                                  kind="ExternalInput").ap(),
        "maag": nc.dram_tensor("maag", [1, D], BF16,
                               kind="ExternalInput").ap(),
        "shiftm": nc.dram_tensor("shiftm", [128, 128], F32,
                                 kind="ExternalInput").ap(),
        "e0": nc.dram_tensor("e0", [1, 128], F32, kind="ExternalInput").ap(),
        "e127": nc.dram_tensor("e127", [128, 128], F32,
                               kind="ExternalInput").ap(),
        "ones": nc.dram_tensor("ones", [128, 128], BF16,
                               kind="ExternalInput").ap(),
        "out": nc.dram_tensor("out", [T, 5, D], F32,
                              kind="ExternalOutput").ap(),
    }
    with tile.TileContext(nc) as tc:
        build_tile_program(tc, io, T, tps)
    nc.compile()
    return nc


_NC_CACHE = {}


def kernel(x, state, time_maa_x, time_maa_w1, time_maa_w2,
           maa_k, maa_w, maa_v, maa_r, maa_g, i):
    global LAST_RESULT
    x = np.asarray(x, np.float32)
    state = np.asarray(state, np.float32)
    B, S, D_ = x.shape
    assert (B, S, D_) == (4, 2048, D)
    T = S * B // N_CORES  # 1024 tokens per core
    i1 = (2 + HEAD) * int(i) + 1

    consts = host_pack_weights(time_maa_x, time_maa_w1, time_maa_w2,
                               maa_k, maa_w, maa_v, maa_r, maa_g)
    in_maps = []
    for c in range(N_CORES):
        b, half = c // 2, c % 2
        xs = np.ascontiguousarray(x[b, half * T:(half + 1) * T])
        prev = state[b, i1] if half == 0 else x[b, T - 1]
        in_maps.append({
            "x": xs,
            "xbf": xs.astype(ml_dtypes.bfloat16),
            "halo0": np.ascontiguousarray(prev[None], np.float32),
            "haloT": np.ascontiguousarray(
                prev.reshape(NK, 128).T).astype(ml_dtypes.bfloat16),
            **consts})

    key = T
    if key not in _NC_CACHE:
        _NC_CACHE[key] = build_nc(T)
    nc = _NC_CACHE[key]

    global _LAST_IN_MAPS
    _LAST_IN_MAPS = in_maps
    LAST_RESULT = run_bass_kernel_spmd(nc, in_maps, list(range(N_CORES)))
    res = LAST_RESULT.results

    out = np.empty((B, S, 5, D), np.float32)
    for c in range(N_CORES):
        b, half = c // 2, c % 2
        out[b, half * T:(half + 1) * T] = res[c]["out"].reshape(T, 5, D)

    new_state = state.copy()
    new_state[:, i1] = x[:, -1]
    return out, new_state
